# revision 6
# baseline (speedup 1.0000x reference)
"""CAGNN layer (edge+node message passing) on 8 trn2 NeuronCores.

Sharding: edges and nodes row-sharded across the 8 cores. The full edge-feat
gather table is assembled on-device by an AllGather of the per-core shards;
new edge features are exchanged with a second AllGather (the halo exchange)
before the node-path gather.

Per-core dataflow (both paths share the same structure):
  indirect-DMA gather of 16 neighbor rows per token (16 x 64KB per 128-token
  block) -> DVE tree-sum over the 16 neighbors
  -> PE transpose to feature-major
  -> matmul [self|neigh-sum] @ W  (neigh half of W pre-scaled by 1/16 on host)
  -> ReLU (+bias) on ACT, second matmul, ReLU
  -> PE transpose back to token-major
  -> LayerNorm (bn_stats/bn_aggr on DVE)
  -> DMA out
"""

import numpy as np

import concourse.bacc as bacc
import concourse.bass as bass
import concourse.tile as tile
from concourse import mybir
from concourse.bass_utils import run_bass_kernel_spmd
from concourse.masks import make_identity

F32 = mybir.dt.float32
I32 = mybir.dt.int32

# Problem geometry (full size; overridable for small-scale testing).
CFG = dict(
    NC=8,          # cores
    E=100000,      # edges
    N=50000,       # nodes
    K=16,          # neighbors
    F=128,         # feature/hidden dim (all of Fn, Fe, H)
)

MAXBLK = 4  # 128-token blocks per superblock (PSUM free-dim limit: 512 f32)


def _geom(cfg):
    NC, E, N, K, F = (cfg[k] for k in ("NC", "E", "N", "K", "F"))
    e_sh = E // NC
    n_sh = N // NC
    e_blk = -(-e_sh // 128)          # 128-token blocks per core, edge path
    n_blk = -(-n_sh // 128)
    return dict(NC=NC, E=E, N=N, K=K, F=F,
                E_SH=e_sh, N_SH=n_sh, E_BLK=e_blk, N_BLK=n_blk,
                E_PAD=e_blk * 128, N_PAD=n_blk * 128)


def _emit_path(nc, tc, pools, *, tbl, idx_sb, x_dram, out_dram,
               w1a, w1b, b1, w2, b2, gam_bc, bet_bc, eps_sb, ident,
               n_blocks, g):
    """Emit one message-passing path (edge or node) over n_blocks blocks of
    128 tokens, grouped into superblocks of up to MAXBLK blocks.

    tbl:    DRAM gather table [T, F]
    idx_sb: resident SBUF index tile [128, n_blocks*K] int32 (partition-major)
    x_dram: self features [n_blocks*128, F] DRAM
    out_dram: output [n_blocks*128, F] DRAM
    """
    K, F = g["K"], g["F"]
    sb_pool, ps_pool, ln_pool = pools["sb"], pools["psum"], pools["ln"]

    x_r = x_dram.rearrange("(j p) f -> p j f", p=128)
    o_r = out_dram.rearrange("(j p) f -> p j f", p=128)
    idx3 = idx_sb.rearrange("p (j k) -> p j k", k=K)

    n_sb = -(-n_blocks // MAXBLK)
    for sb in range(n_sb):
        nb = min(MAXBLK, n_blocks - sb * MAXBLK)
        W = nb * F  # free width this superblock

        # ---- neighbor gather + tree-sum -> s4 [128, W] (sum of K rows)
        s4 = sb_pool.tile([128, MAXBLK * F], F32, tag="s4")
        for b in range(nb):
            j = sb * MAXBLK + b
            gt = sb_pool.tile([128, K * F], F32, tag="gather", bufs=6)
            for k in range(K):
                nc.gpsimd.indirect_dma_start(
                    out=gt[:, k * F:(k + 1) * F],
                    out_offset=None,
                    in_=tbl[:],
                    in_offset=bass.IndirectOffsetOnAxis(
                        ap=idx3[:, j, k:k + 1], axis=0),
                )
            w = K * F // 2
            while w >= F:
                lo = gt[:, :w]
                hi = gt[:, w:2 * w]
                if w == F:
                    nc.vector.tensor_add(s4[:, b * F:(b + 1) * F], lo, hi)
                else:
                    nc.vector.tensor_add(lo, lo, hi)
                w //= 2

        # ---- self features
        x4 = sb_pool.tile([128, MAXBLK, F], F32, tag="x4")
        nc.sync.dma_start(out=x4[:, :nb, :],
                          in_=x_r[:, sb * MAXBLK:sb * MAXBLK + nb, :])

        # ---- transpose self + neigh-sum to feature-major
        tp_x = ps_pool.tile([128, MAXBLK * F], F32, tag="tp", space="PSUM")
        for b in range(nb):
            nc.tensor.transpose(out=tp_x[:, b * F:(b + 1) * F],
                                in_=x4[:, b, :], identity=ident)
        xt = sb_pool.tile([128, MAXBLK * F], F32, tag="xt")
        nc.scalar.activation(out=xt[:, :W], in_=tp_x[:, :W],
                             func=mybir.ActivationFunctionType.Copy)

        tp_s = ps_pool.tile([128, MAXBLK * F], F32, tag="tp", space="PSUM")
        for b in range(nb):
            nc.tensor.transpose(out=tp_s[:, b * F:(b + 1) * F],
                                in_=s4[:, b * F:(b + 1) * F], identity=ident)
        st = sb_pool.tile([128, MAXBLK * F], F32, tag="st")
        nc.scalar.activation(out=st[:, :W], in_=tp_s[:, :W],
                             func=mybir.ActivationFunctionType.Copy)

        # ---- h1 = relu(W1a.T @ xt + W1b.T @ st + b1)   [H, W]
        ph1 = ps_pool.tile([128, MAXBLK * F], F32, tag="h", space="PSUM")
        nc.tensor.matmul(ph1[:, :W], w1a[:], xt[:, :W], start=True, stop=False)
        nc.tensor.matmul(ph1[:, :W], w1b[:], st[:, :W], start=False, stop=True)
        h1 = sb_pool.tile([128, MAXBLK * F], F32, tag="h1")
        nc.scalar.activation(out=h1[:, :W], in_=ph1[:, :W],
                             func=mybir.ActivationFunctionType.Relu,
                             bias=b1[:, 0:1], scale=1.0)

        # ---- h2 = relu(W2.T @ h1 + b2)   [H, W]
        ph2 = ps_pool.tile([128, MAXBLK * F], F32, tag="h", space="PSUM")
        nc.tensor.matmul(ph2[:, :W], w2[:], h1[:, :W], start=True, stop=True)
        h2 = sb_pool.tile([128, MAXBLK * F], F32, tag="h2")
        nc.scalar.activation(out=h2[:, :W], in_=ph2[:, :W],
                             func=mybir.ActivationFunctionType.Relu,
                             bias=b2[:, 0:1], scale=1.0)

        # ---- transpose back to token-major
        tp_y = ps_pool.tile([128, MAXBLK * F], F32, tag="y", space="PSUM")
        for b in range(nb):
            nc.tensor.transpose(out=tp_y[:, b * F:(b + 1) * F],
                                in_=h2[:, b * F:(b + 1) * F], identity=ident)
        y4 = sb_pool.tile([128, MAXBLK * F], F32, tag="y4")
        nc.scalar.activation(out=y4[:, :W], in_=tp_y[:, :W],
                             func=mybir.ActivationFunctionType.Copy)

        # ---- LayerNorm per 128-token block over F
        for b in range(nb):
            yb = y4[:, b * F:(b + 1) * F]
            stats = ln_pool.tile([128, 6], F32, tag="stats")
            nc.vector.bn_stats(out=stats[:], in_=yb)
            mv = ln_pool.tile([128, 2], F32, tag="mv")
            nc.vector.bn_aggr(out=mv[:], in_=stats[:])
            nc.scalar.activation(out=mv[:, 1:2], in_=mv[:, 1:2],
                                 func=mybir.ActivationFunctionType.Sqrt,
                                 bias=eps_sb[:, 0:1], scale=1.0)
            nc.vector.reciprocal(out=mv[:, 1:2], in_=mv[:, 1:2])
            nc.vector.tensor_scalar(out=yb, in0=yb,
                                    scalar1=mv[:, 0:1], scalar2=mv[:, 1:2],
                                    op0=mybir.AluOpType.subtract,
                                    op1=mybir.AluOpType.mult)
            nc.vector.tensor_mul(yb, yb, gam_bc[:])
            nc.vector.tensor_add(yb, yb, bet_bc[:])

        nc.sync.dma_start(
            out=o_r[:, sb * MAXBLK:sb * MAXBLK + nb, :],
            in_=y4[:, :W].rearrange("p (b f) -> p b f", f=F))


def build_nc(cfg=None):
    g = _geom(cfg or CFG)
    NC, E, K, F = g["NC"], g["E"], g["K"], g["F"]

    nc = bacc.Bacc("TRN2", target_bir_lowering=False, debug=False,
                   enable_asserts=False, num_devices=NC)

    # --- external I/O (per core)
    eidx = nc.dram_tensor("eidx", [128, g["E_BLK"] * K], I32,
                          kind="ExternalInput").ap()
    nidx = nc.dram_tensor("nidx", [128, g["N_BLK"] * K], I32,
                          kind="ExternalInput").ap()
    xe = nc.dram_tensor("xe", [g["E_PAD"], F], F32, kind="ExternalInput").ap()
    xn = nc.dram_tensor("xn", [g["N_PAD"], F], F32, kind="ExternalInput").ap()
    wts = {}
    for nm, shp in [("we1a", [F, F]), ("we1b", [F, F]), ("be1", [F]),
                    ("we2", [F, F]), ("be2", [F]),
                    ("wn1a", [F, F]), ("wn1b", [F, F]), ("bn1", [F]),
                    ("wn2", [F, F]), ("bn2", [F]),
                    ("gam", [F]), ("bet", [F])]:
        wts[nm] = nc.dram_tensor(nm, shp, F32, kind="ExternalInput").ap()
    oute = nc.dram_tensor("oute", [g["E_PAD"], F], F32,
                          kind="ExternalOutput").ap()
    outn = nc.dram_tensor("outn", [g["N_PAD"], F], F32,
                          kind="ExternalOutput").ap()

    with tile.TileContext(nc) as tc:
        with (
            tc.tile_pool(name="const", bufs=1) as const_pool,
            tc.tile_pool(name="sb", bufs=3) as sb_pool,
            tc.tile_pool(name="psum", bufs=2, space="PSUM") as ps_pool,
            tc.tile_pool(name="ln", bufs=4) as ln_pool,
            tc.tile_pool(name="dram", bufs=1, space="DRAM") as dram_pool,
        ):
            pools = dict(sb=sb_pool, psum=ps_pool, ln=ln_pool)

            # --- one-time constants
            ident = const_pool.tile([128, 128], F32)
            make_identity(nc, ident[:])
            eps_sb = const_pool.tile([128, 1], F32)
            nc.vector.memset(eps_sb[:], 1e-5)

            def load_w(name):
                t = const_pool.tile([F, F], F32, name=f"sb_{name}")
                nc.sync.dma_start(out=t[:], in_=wts[name][:])
                return t

            def load_b(name):
                t = const_pool.tile([128, 1], F32, name=f"sb_{name}")
                nc.sync.dma_start(out=t[:], in_=wts[name][:, None])
                return t

            def load_bc(name):
                t = const_pool.tile([128, F], F32, name=f"sb_{name}")
                src = bass.AP(tensor=wts[name].tensor, offset=0,
                              ap=[[0, 128], [1, F]])
                nc.gpsimd.dma_start(out=t[:], in_=src)
                return t

            we1a, we1b, we2 = load_w("we1a"), load_w("we1b"), load_w("we2")
            wn1a, wn1b, wn2 = load_w("wn1a"), load_w("wn1b"), load_w("wn2")
            be1, be2 = load_b("be1"), load_b("be2")
            bn1, bn2 = load_b("bn1"), load_b("bn2")
            gam_bc, bet_bc = load_bc("gam"), load_bc("bet")

            # --- resident index tiles (host pre-shuffled to partition-major)
            eidx_sb = const_pool.tile([128, g["E_BLK"] * K], I32)
            nc.sync.dma_start(out=eidx_sb[:], in_=eidx[:])
            nidx_sb = const_pool.tile([128, g["N_BLK"] * K], I32)
            nc.sync.dma_start(out=nidx_sb[:], in_=nidx[:])

            # --- internal DRAM
            exe_loc = dram_pool.tile([g["E_SH"], F], F32)
            tbl_e = dram_pool.tile([E, F], F32, addr_space="Shared")
            eloc = dram_pool.tile([g["E_PAD"], F], F32)
            tbl_new = dram_pool.tile([E, F], F32, addr_space="Shared")

            # --- assemble the full edge-feat table on device (input halo)
            nc.sync.dma_start(out=exe_loc[:], in_=xe[:g["E_SH"], :])
            nc.gpsimd.collective_compute(
                "AllGather", mybir.AluOpType.bypass,
                replica_groups=[list(range(NC))],
                ins=[exe_loc[:].opt()],
                outs=[tbl_e[:].opt()],
            )

            # --- edge path
            _emit_path(nc, tc, pools, tbl=tbl_e[:], idx_sb=eidx_sb[:],
                       x_dram=xe, out_dram=eloc[:],
                       w1a=we1a, w1b=we1b, b1=be1, w2=we2, b2=be2,
                       gam_bc=gam_bc, bet_bc=bet_bc, eps_sb=eps_sb,
                       ident=ident, n_blocks=g["E_BLK"], g=g)

            # copy local shard to external output (overlaps with node path)
            nc.sync.dma_start(out=oute[:g["E_SH"], :],
                              in_=eloc[:g["E_SH"], :])

            # --- all-gather the new edge features across the 8 cores
            nc.gpsimd.collective_compute(
                "AllGather", mybir.AluOpType.bypass,
                replica_groups=[list(range(NC))],
                ins=[eloc[:g["E_SH"], :].opt()],
                outs=[tbl_new[:].opt()],
            )

            # --- node path (gathers from the all-gathered table)
            _emit_path(nc, tc, pools, tbl=tbl_new[:], idx_sb=nidx_sb[:],
                       x_dram=xn, out_dram=outn,
                       w1a=wn1a, w1b=wn1b, b1=bn1, w2=wn2, b2=bn2,
                       gam_bc=gam_bc, bet_bc=bet_bc, eps_sb=eps_sb,
                       ident=ident, n_blocks=g["N_BLK"], g=g)

    nc.finalize()
    return nc, g


_BUILT = None


def _shuffle_idx(idx, g):
    """[rows, K] int32 -> [128, nblk*K] partition-major for the gather AP."""
    nblk = idx.shape[0] // 128
    return (idx.reshape(nblk, 128, g["K"]).transpose(1, 0, 2)
            .reshape(128, nblk * g["K"]).copy())


def _pad_rows(a, n):
    if a.shape[0] == n:
        return np.ascontiguousarray(a)
    out = np.zeros((n,) + a.shape[1:], a.dtype)
    out[:a.shape[0]] = a
    return out


def _make_in_maps(inputs, g):
    K, F = g["K"], g["F"]
    f32 = lambda x: np.ascontiguousarray(np.asarray(x), dtype=np.float32)
    i32 = lambda x: np.ascontiguousarray(np.asarray(x), dtype=np.int32)

    e_nb = i32(inputs["edge_neighbors"])
    n_nb = i32(inputs["node_neighbors"])
    ef = f32(inputs["edge_feats"])
    nf = f32(inputs["node_feats"])

    w_e1 = f32(inputs["W_edge_agg"])
    w_n1 = f32(inputs["W_node_agg"])
    shared = dict(
        we1a=np.ascontiguousarray(w_e1[:F]),
        we1b=np.ascontiguousarray(w_e1[F:] / K),
        be1=f32(inputs["b_edge_agg"]),
        we2=f32(inputs["W_edge_com"]), be2=f32(inputs["b_edge_com"]),
        wn1a=np.ascontiguousarray(w_n1[:F]),
        wn1b=np.ascontiguousarray(w_n1[F:] / K),
        bn1=f32(inputs["b_node_agg"]),
        wn2=f32(inputs["W_node_com"]), bn2=f32(inputs["b_node_com"]),
        gam=f32(inputs["ln_gamma"]), bet=f32(inputs["ln_beta"]),
    )

    in_maps = []
    for c in range(g["NC"]):
        esl = slice(c * g["E_SH"], (c + 1) * g["E_SH"])
        nsl = slice(c * g["N_SH"], (c + 1) * g["N_SH"])
        m = dict(shared)
        m["eidx"] = _shuffle_idx(_pad_rows(e_nb[esl], g["E_PAD"]), g)
        m["nidx"] = _shuffle_idx(_pad_rows(n_nb[nsl], g["N_PAD"]), g)
        m["xe"] = _pad_rows(ef[esl], g["E_PAD"])
        m["xn"] = _pad_rows(nf[nsl], g["N_PAD"])
        in_maps.append(m)
    return in_maps


def _run(inputs, cfg=None, trace=False):
    global _BUILT
    cfg = cfg or CFG
    if _BUILT is None or _BUILT[2] != cfg:
        nc, g = build_nc(cfg)
        _BUILT = (nc, g, dict(cfg))
    nc, g, _ = _BUILT
    NC = g["NC"]

    in_maps = _make_in_maps(inputs, g)
    res = run_bass_kernel_spmd(nc, in_maps, list(range(NC)), trace=trace)

    new_edge = np.concatenate(
        [res.results[c]["oute"][:g["E_SH"]] for c in range(NC)], axis=0)
    new_node = np.concatenate(
        [res.results[c]["outn"][:g["N_SH"]] for c in range(NC)], axis=0)
    return (new_node, new_edge), res


def kernel(**inputs):
    (new_node, new_edge), _ = _run(inputs)
    return new_node, new_edge


# revision 7
# speedup vs baseline: 1.6128x; 1.6128x over previous
"""CAGNN layer (edge+node message passing) on 8 trn2 NeuronCores.

Sharding: edges and nodes row-sharded across the 8 cores. The full edge-feat
gather table is assembled on-device by an AllGather of the per-core shards;
new edge features are exchanged with a second AllGather (the halo exchange)
before the node-path gather.

Per-core dataflow (both paths share the same structure):
  indirect-DMA gather of 16 neighbor rows per token (16 x 64KB per 128-token
  block) -> DVE tree-sum over the 16 neighbors
  -> PE transpose to feature-major
  -> matmul [self|neigh-sum] @ W  (neigh half of W pre-scaled by 1/16 on host)
  -> ReLU (+bias) on ACT, second matmul, ReLU
  -> PE transpose back to token-major
  -> LayerNorm (bn_stats/bn_aggr on DVE)
  -> DMA out
"""

import numpy as np

import concourse.bacc as bacc
import concourse.bass as bass
import concourse.tile as tile
from concourse import mybir
from concourse.bass_utils import run_bass_kernel_spmd
from concourse.masks import make_identity

F32 = mybir.dt.float32
I32 = mybir.dt.int32

# Problem geometry (full size; overridable for small-scale testing).
CFG = dict(
    NC=8,          # cores
    E=100000,      # edges
    N=50000,       # nodes
    K=16,          # neighbors
    F=128,         # feature/hidden dim (all of Fn, Fe, H)
)

MAXBLK = 4  # 128-token blocks per superblock (PSUM free-dim limit: 512 f32)


def _geom(cfg):
    NC, E, N, K, F = (cfg[k] for k in ("NC", "E", "N", "K", "F"))
    e_sh = E // NC
    n_sh = N // NC
    e_blk = -(-e_sh // 128)          # 128-token blocks per core, edge path
    n_blk = -(-n_sh // 128)
    return dict(NC=NC, E=E, N=N, K=K, F=F,
                E_SH=e_sh, N_SH=n_sh, E_BLK=e_blk, N_BLK=n_blk,
                E_PAD=e_blk * 128, N_PAD=n_blk * 128)


def _emit_path(nc, tc, pools, *, tbl, idx_sb, x_dram, out_dram,
               w1a, w1b, b1, w2, b2, gam_bc, bet_bc, eps_sb, ident,
               n_blocks, g):
    """Emit one message-passing path (edge or node) over n_blocks blocks of
    128 tokens, grouped into superblocks of up to MAXBLK blocks.

    tbl:    DRAM gather table [T, F]
    idx_sb: resident SBUF index tile [128, n_blocks*K] int32 (partition-major)
    x_dram: self features [n_blocks*128, F] DRAM
    out_dram: output [n_blocks*128, F] DRAM
    """
    K, F = g["K"], g["F"]
    sb_pool, ps_pool, ln_pool = pools["sb"], pools["psum"], pools["ln"]

    x_r = x_dram.rearrange("(j p) f -> p j f", p=128)
    o_r = out_dram.rearrange("(j p) f -> p j f", p=128)
    idx3 = idx_sb.rearrange("p (j k) -> p j k", k=K)

    n_sb = -(-n_blocks // MAXBLK)
    for sb in range(n_sb):
        nb = min(MAXBLK, n_blocks - sb * MAXBLK)
        W = nb * F  # free width this superblock

        # ---- neighbor gather + tree-sum -> s4 [128, W] (sum of K rows)
        s4 = sb_pool.tile([128, MAXBLK * F], F32, tag="s4")
        for b in range(nb):
            j = sb * MAXBLK + b
            gt = sb_pool.tile([128, K * F], F32, tag="gather", bufs=6)
            for k in range(K):
                nc.gpsimd.indirect_dma_start(
                    out=gt[:, k * F:(k + 1) * F],
                    out_offset=None,
                    in_=tbl[:],
                    in_offset=bass.IndirectOffsetOnAxis(
                        ap=idx3[:, j, k:k + 1], axis=0),
                )
            w = K * F // 2
            while w >= F:
                lo = gt[:, :w]
                hi = gt[:, w:2 * w]
                if w == F:
                    nc.vector.tensor_add(s4[:, b * F:(b + 1) * F], lo, hi)
                else:
                    nc.vector.tensor_add(lo, lo, hi)
                w //= 2

        # ---- self features
        x4 = sb_pool.tile([128, MAXBLK, F], F32, tag="x4")
        nc.sync.dma_start(out=x4[:, :nb, :],
                          in_=x_r[:, sb * MAXBLK:sb * MAXBLK + nb, :])

        # ---- transpose self + neigh-sum to feature-major
        tp_x = ps_pool.tile([128, MAXBLK * F], F32, tag="tp", space="PSUM")
        for b in range(nb):
            nc.tensor.transpose(out=tp_x[:, b * F:(b + 1) * F],
                                in_=x4[:, b, :], identity=ident)
        xt = sb_pool.tile([128, MAXBLK * F], F32, tag="xt")
        nc.scalar.activation(out=xt[:, :W], in_=tp_x[:, :W],
                             func=mybir.ActivationFunctionType.Copy)

        tp_s = ps_pool.tile([128, MAXBLK * F], F32, tag="tp", space="PSUM")
        for b in range(nb):
            nc.tensor.transpose(out=tp_s[:, b * F:(b + 1) * F],
                                in_=s4[:, b * F:(b + 1) * F], identity=ident)
        st = sb_pool.tile([128, MAXBLK * F], F32, tag="st")
        nc.scalar.activation(out=st[:, :W], in_=tp_s[:, :W],
                             func=mybir.ActivationFunctionType.Copy)

        # ---- h1 = relu(W1a.T @ xt + W1b.T @ st + b1)   [H, W]
        ph1 = ps_pool.tile([128, MAXBLK * F], F32, tag="h", space="PSUM")
        nc.tensor.matmul(ph1[:, :W], w1a[:], xt[:, :W], start=True, stop=False)
        nc.tensor.matmul(ph1[:, :W], w1b[:], st[:, :W], start=False, stop=True)
        h1 = sb_pool.tile([128, MAXBLK * F], F32, tag="h1")
        nc.scalar.activation(out=h1[:, :W], in_=ph1[:, :W],
                             func=mybir.ActivationFunctionType.Relu,
                             bias=b1[:, 0:1], scale=1.0)

        # ---- h2 = relu(W2.T @ h1 + b2)   [H, W]
        ph2 = ps_pool.tile([128, MAXBLK * F], F32, tag="h", space="PSUM")
        nc.tensor.matmul(ph2[:, :W], w2[:], h1[:, :W], start=True, stop=True)
        h2 = sb_pool.tile([128, MAXBLK * F], F32, tag="h2")
        nc.scalar.activation(out=h2[:, :W], in_=ph2[:, :W],
                             func=mybir.ActivationFunctionType.Relu,
                             bias=b2[:, 0:1], scale=1.0)

        # ---- transpose back to token-major
        tp_y = ps_pool.tile([128, MAXBLK * F], F32, tag="y", space="PSUM")
        for b in range(nb):
            nc.tensor.transpose(out=tp_y[:, b * F:(b + 1) * F],
                                in_=h2[:, b * F:(b + 1) * F], identity=ident)
        y4 = sb_pool.tile([128, MAXBLK * F], F32, tag="y4")
        nc.scalar.activation(out=y4[:, :W], in_=tp_y[:, :W],
                             func=mybir.ActivationFunctionType.Copy)

        # ---- LayerNorm per 128-token block over F
        for b in range(nb):
            yb = y4[:, b * F:(b + 1) * F]
            stats = ln_pool.tile([128, 6], F32, tag="stats")
            nc.vector.bn_stats(out=stats[:], in_=yb)
            mv = ln_pool.tile([128, 2], F32, tag="mv")
            nc.vector.bn_aggr(out=mv[:], in_=stats[:])
            nc.scalar.activation(out=mv[:, 1:2], in_=mv[:, 1:2],
                                 func=mybir.ActivationFunctionType.Sqrt,
                                 bias=eps_sb[:, 0:1], scale=1.0)
            nc.vector.reciprocal(out=mv[:, 1:2], in_=mv[:, 1:2])
            nc.vector.tensor_scalar(out=yb, in0=yb,
                                    scalar1=mv[:, 0:1], scalar2=mv[:, 1:2],
                                    op0=mybir.AluOpType.subtract,
                                    op1=mybir.AluOpType.mult)
            nc.vector.tensor_mul(yb, yb, gam_bc[:])
            nc.vector.tensor_add(yb, yb, bet_bc[:])

        nc.sync.dma_start(
            out=o_r[:, sb * MAXBLK:sb * MAXBLK + nb, :],
            in_=y4[:, :W].rearrange("p (b f) -> p b f", f=F))


def build_nc(cfg=None):
    g = _geom(cfg or CFG)
    NC, E, K, F = g["NC"], g["E"], g["K"], g["F"]

    nc = bacc.Bacc("TRN2", target_bir_lowering=False, debug=False,
                   enable_asserts=False, num_devices=NC)

    # --- external I/O (per core)
    eidx = nc.dram_tensor("eidx", [128, g["E_BLK"] * K], I32,
                          kind="ExternalInput").ap()
    nidx = nc.dram_tensor("nidx", [128, g["N_BLK"] * K], I32,
                          kind="ExternalInput").ap()
    xe = nc.dram_tensor("xe", [g["E_PAD"], F], F32, kind="ExternalInput").ap()
    xn = nc.dram_tensor("xn", [g["N_PAD"], F], F32, kind="ExternalInput").ap()
    wts = {}
    for nm, shp in [("we1a", [F, F]), ("we1b", [F, F]), ("be1", [F]),
                    ("we2", [F, F]), ("be2", [F]),
                    ("wn1a", [F, F]), ("wn1b", [F, F]), ("bn1", [F]),
                    ("wn2", [F, F]), ("bn2", [F]),
                    ("gam", [F]), ("bet", [F])]:
        wts[nm] = nc.dram_tensor(nm, shp, F32, kind="ExternalInput").ap()
    oute = nc.dram_tensor("oute", [g["E_PAD"], F], F32,
                          kind="ExternalOutput").ap()
    outn = nc.dram_tensor("outn", [g["N_PAD"], F], F32,
                          kind="ExternalOutput").ap()

    with tile.TileContext(nc) as tc:
        with (
            tc.tile_pool(name="const", bufs=1) as const_pool,
            tc.tile_pool(name="sb", bufs=2) as sb_pool,
            tc.tile_pool(name="psum", bufs=2, space="PSUM") as ps_pool,
            tc.tile_pool(name="ln", bufs=4) as ln_pool,
            tc.tile_pool(name="dram", bufs=1, space="DRAM") as dram_pool,
        ):
            pools = dict(sb=sb_pool, psum=ps_pool, ln=ln_pool)

            # --- one-time constants
            ident = const_pool.tile([128, 128], F32)
            make_identity(nc, ident[:])
            eps_sb = const_pool.tile([128, 1], F32)
            nc.vector.memset(eps_sb[:], 1e-5)

            def load_w(name):
                t = const_pool.tile([F, F], F32, name=f"sb_{name}")
                nc.sync.dma_start(out=t[:], in_=wts[name][:])
                return t

            def load_b(name):
                t = const_pool.tile([128, 1], F32, name=f"sb_{name}")
                nc.sync.dma_start(out=t[:], in_=wts[name][:, None])
                return t

            def load_bc(name):
                t = const_pool.tile([128, F], F32, name=f"sb_{name}")
                src = bass.AP(tensor=wts[name].tensor, offset=0,
                              ap=[[0, 128], [1, F]])
                nc.gpsimd.dma_start(out=t[:], in_=src)
                return t

            we1a, we1b, we2 = load_w("we1a"), load_w("we1b"), load_w("we2")
            wn1a, wn1b, wn2 = load_w("wn1a"), load_w("wn1b"), load_w("wn2")
            be1, be2 = load_b("be1"), load_b("be2")
            bn1, bn2 = load_b("bn1"), load_b("bn2")
            gam_bc, bet_bc = load_bc("gam"), load_bc("bet")

            # --- resident index tiles (host pre-shuffled to partition-major)
            eidx_sb = const_pool.tile([128, g["E_BLK"] * K], I32)
            nc.sync.dma_start(out=eidx_sb[:], in_=eidx[:])
            nidx_sb = const_pool.tile([128, g["N_BLK"] * K], I32)
            nc.sync.dma_start(out=nidx_sb[:], in_=nidx[:])

            # --- internal DRAM
            exe_loc = dram_pool.tile([g["E_SH"], F], F32)
            tbl_e = dram_pool.tile([E, F], F32, addr_space="Shared")
            eloc = dram_pool.tile([g["E_PAD"], F], F32)
            tbl_new = dram_pool.tile([E, F], F32, addr_space="Shared")

            # --- assemble the full edge-feat table on device (input halo)
            nc.sync.dma_start(out=exe_loc[:], in_=xe[:g["E_SH"], :])
            nc.gpsimd.collective_compute(
                "AllGather", mybir.AluOpType.bypass,
                replica_groups=[list(range(NC))],
                ins=[exe_loc[:].opt()],
                outs=[tbl_e[:].opt()],
            )

            # --- edge path
            _emit_path(nc, tc, pools, tbl=tbl_e[:], idx_sb=eidx_sb[:],
                       x_dram=xe, out_dram=eloc[:],
                       w1a=we1a, w1b=we1b, b1=be1, w2=we2, b2=be2,
                       gam_bc=gam_bc, bet_bc=bet_bc, eps_sb=eps_sb,
                       ident=ident, n_blocks=g["E_BLK"], g=g)

            # copy local shard to external output (overlaps with node path)
            nc.sync.dma_start(out=oute[:g["E_SH"], :],
                              in_=eloc[:g["E_SH"], :])

            # --- all-gather the new edge features across the 8 cores
            nc.gpsimd.collective_compute(
                "AllGather", mybir.AluOpType.bypass,
                replica_groups=[list(range(NC))],
                ins=[eloc[:g["E_SH"], :].opt()],
                outs=[tbl_new[:].opt()],
            )

            # --- node path (gathers from the all-gathered table)
            _emit_path(nc, tc, pools, tbl=tbl_new[:], idx_sb=nidx_sb[:],
                       x_dram=xn, out_dram=outn,
                       w1a=wn1a, w1b=wn1b, b1=bn1, w2=wn2, b2=bn2,
                       gam_bc=gam_bc, bet_bc=bet_bc, eps_sb=eps_sb,
                       ident=ident, n_blocks=g["N_BLK"], g=g)

    nc.finalize()
    return nc, g


_BUILT = None


def _shuffle_idx(idx, g):
    """[rows, K] int32 -> [128, nblk*K] partition-major for the gather AP."""
    nblk = idx.shape[0] // 128
    return (idx.reshape(nblk, 128, g["K"]).transpose(1, 0, 2)
            .reshape(128, nblk * g["K"]).copy())


def _pad_rows(a, n):
    if a.shape[0] == n:
        return np.ascontiguousarray(a)
    out = np.zeros((n,) + a.shape[1:], a.dtype)
    out[:a.shape[0]] = a
    return out


def _make_in_maps(inputs, g):
    K, F = g["K"], g["F"]
    f32 = lambda x: np.ascontiguousarray(np.asarray(x), dtype=np.float32)
    i32 = lambda x: np.ascontiguousarray(np.asarray(x), dtype=np.int32)

    e_nb = i32(inputs["edge_neighbors"])
    n_nb = i32(inputs["node_neighbors"])
    ef = f32(inputs["edge_feats"])
    nf = f32(inputs["node_feats"])

    w_e1 = f32(inputs["W_edge_agg"])
    w_n1 = f32(inputs["W_node_agg"])
    shared = dict(
        we1a=np.ascontiguousarray(w_e1[:F]),
        we1b=np.ascontiguousarray(w_e1[F:] / K),
        be1=f32(inputs["b_edge_agg"]),
        we2=f32(inputs["W_edge_com"]), be2=f32(inputs["b_edge_com"]),
        wn1a=np.ascontiguousarray(w_n1[:F]),
        wn1b=np.ascontiguousarray(w_n1[F:] / K),
        bn1=f32(inputs["b_node_agg"]),
        wn2=f32(inputs["W_node_com"]), bn2=f32(inputs["b_node_com"]),
        gam=f32(inputs["ln_gamma"]), bet=f32(inputs["ln_beta"]),
    )

    in_maps = []
    for c in range(g["NC"]):
        esl = slice(c * g["E_SH"], (c + 1) * g["E_SH"])
        nsl = slice(c * g["N_SH"], (c + 1) * g["N_SH"])
        m = dict(shared)
        m["eidx"] = _shuffle_idx(_pad_rows(e_nb[esl], g["E_PAD"]), g)
        m["nidx"] = _shuffle_idx(_pad_rows(n_nb[nsl], g["N_PAD"]), g)
        m["xe"] = _pad_rows(ef[esl], g["E_PAD"])
        m["xn"] = _pad_rows(nf[nsl], g["N_PAD"])
        in_maps.append(m)
    return in_maps


def _run(inputs, cfg=None, trace=False):
    global _BUILT
    cfg = cfg or CFG
    if _BUILT is None or _BUILT[2] != cfg:
        nc, g = build_nc(cfg)
        _BUILT = (nc, g, dict(cfg))
    nc, g, _ = _BUILT
    NC = g["NC"]

    in_maps = _make_in_maps(inputs, g)
    res = run_bass_kernel_spmd(nc, in_maps, list(range(NC)), trace=trace)

    new_edge = np.concatenate(
        [res.results[c]["oute"][:g["E_SH"]] for c in range(NC)], axis=0)
    new_node = np.concatenate(
        [res.results[c]["outn"][:g["N_SH"]] for c in range(NC)], axis=0)
    return (new_node, new_edge), res


def kernel(**inputs):
    (new_node, new_edge), _ = _run(inputs)
    return new_node, new_edge


# revision 8
# speedup vs baseline: 1.8478x; 1.1457x over previous
"""CAGNN layer (edge+node message passing) on 8 trn2 NeuronCores.

Sharding: edges and nodes row-sharded across the 8 cores. The full edge-feat
gather table is assembled on-device by an AllGather of the per-core shards;
new edge features are exchanged with a second AllGather (the halo exchange)
before the node-path gather.

Per-core dataflow (both paths share the same structure):
  indirect-DMA gather of 16 neighbor rows per token (16 x 64KB per 128-token
  block) -> DVE tree-sum over the 16 neighbors
  -> PE transpose to feature-major
  -> matmul [self|neigh-sum] @ W  (neigh half of W pre-scaled by 1/16 on host)
  -> ReLU (+bias) on ACT, second matmul, ReLU
  -> PE transpose back to token-major
  -> LayerNorm (bn_stats/bn_aggr on DVE)
  -> DMA out
"""

import numpy as np

import concourse.bacc as bacc
import concourse.bass as bass
import concourse.tile as tile
from concourse import mybir
from concourse.bass_utils import run_bass_kernel_spmd
from concourse.masks import make_identity

F32 = mybir.dt.float32
I32 = mybir.dt.int32

# Problem geometry (full size; overridable for small-scale testing).
CFG = dict(
    NC=8,          # cores
    E=100000,      # edges
    N=50000,       # nodes
    K=16,          # neighbors
    F=128,         # feature/hidden dim (all of Fn, Fe, H)
)

MAXBLK = 4  # 128-token blocks per superblock (PSUM free-dim limit: 512 f32)


def _geom(cfg):
    NC, E, N, K, F = (cfg[k] for k in ("NC", "E", "N", "K", "F"))
    e_sh = E // NC
    n_sh = N // NC
    e_blk = -(-e_sh // 128)          # 128-token blocks per core, edge path
    n_blk = -(-n_sh // 128)
    return dict(NC=NC, E=E, N=N, K=K, F=F,
                E_SH=e_sh, N_SH=n_sh, E_BLK=e_blk, N_BLK=n_blk,
                E_PAD=e_blk * 128, N_PAD=n_blk * 128)


def _emit_path(nc, tc, pools, *, tbl, idx_sb, x_dram, out_dram,
               w1a, w1b, b1, w2, b2, gam_bc, bet_bc, eps_sb, ident,
               n_blocks, g):
    """Emit one message-passing path (edge or node) over n_blocks blocks of
    128 tokens, grouped into superblocks of up to MAXBLK blocks.

    tbl:    DRAM gather table [T, F]
    idx_sb: resident SBUF index tile [128, n_blocks*K] int32 (partition-major)
    x_dram: self features [n_blocks*128, F] DRAM
    out_dram: output [n_blocks*128, F] DRAM
    """
    K, F = g["K"], g["F"]
    sb_pool, ps_pool, ln_pool = pools["sb"], pools["psum"], pools["ln"]

    x_r = x_dram.rearrange("(j p) f -> p j f", p=128)
    o_r = out_dram.rearrange("(j p) f -> p j f", p=128)
    idx3 = idx_sb.rearrange("p (j k) -> p j k", k=K)

    n_sb = -(-n_blocks // MAXBLK)
    for sb in range(n_sb):
        nb = min(MAXBLK, n_blocks - sb * MAXBLK)
        W = nb * F  # free width this superblock

        # ---- neighbor gather + tree-sum -> s4 [128, W] (sum of K rows)
        s4 = sb_pool.tile([128, MAXBLK * F], F32, tag="s4")
        for b in range(nb):
            j = sb * MAXBLK + b
            gt = sb_pool.tile([128, K * F], F32, tag="gather", bufs=6)
            for k in range(K):
                nc.gpsimd.indirect_dma_start(
                    out=gt[:, k * F:(k + 1) * F],
                    out_offset=None,
                    in_=tbl[:],
                    in_offset=bass.IndirectOffsetOnAxis(
                        ap=idx3[:, j, k:k + 1], axis=0),
                )
            w = K * F // 2
            while w >= F:
                lo = gt[:, :w]
                hi = gt[:, w:2 * w]
                if w == F:
                    nc.vector.tensor_add(s4[:, b * F:(b + 1) * F], lo, hi)
                else:
                    nc.vector.tensor_add(lo, lo, hi)
                w //= 2

        # ---- self features
        x4 = sb_pool.tile([128, MAXBLK, F], F32, tag="x4")
        nc.sync.dma_start(out=x4[:, :nb, :],
                          in_=x_r[:, sb * MAXBLK:sb * MAXBLK + nb, :])

        # ---- transpose self + neigh-sum to feature-major
        tp_x = ps_pool.tile([128, MAXBLK * F], F32, tag="tp", space="PSUM")
        for b in range(nb):
            nc.tensor.transpose(out=tp_x[:, b * F:(b + 1) * F],
                                in_=x4[:, b, :], identity=ident)
        xt = sb_pool.tile([128, MAXBLK * F], F32, tag="xt")
        nc.scalar.activation(out=xt[:, :W], in_=tp_x[:, :W],
                             func=mybir.ActivationFunctionType.Copy)

        tp_s = ps_pool.tile([128, MAXBLK * F], F32, tag="tp", space="PSUM")
        for b in range(nb):
            nc.tensor.transpose(out=tp_s[:, b * F:(b + 1) * F],
                                in_=s4[:, b * F:(b + 1) * F], identity=ident)
        st = sb_pool.tile([128, MAXBLK * F], F32, tag="st")
        nc.scalar.activation(out=st[:, :W], in_=tp_s[:, :W],
                             func=mybir.ActivationFunctionType.Copy)

        # ---- h1 = relu(W1a.T @ xt + W1b.T @ st + b1)   [H, W]
        ph1 = ps_pool.tile([128, MAXBLK * F], F32, tag="h", space="PSUM")
        nc.tensor.matmul(ph1[:, :W], w1a[:], xt[:, :W], start=True, stop=False)
        nc.tensor.matmul(ph1[:, :W], w1b[:], st[:, :W], start=False, stop=True)
        h1 = sb_pool.tile([128, MAXBLK * F], F32, tag="h1")
        nc.scalar.activation(out=h1[:, :W], in_=ph1[:, :W],
                             func=mybir.ActivationFunctionType.Relu,
                             bias=b1[:, 0:1], scale=1.0)

        # ---- h2 = relu(W2.T @ h1 + b2)   [H, W]
        ph2 = ps_pool.tile([128, MAXBLK * F], F32, tag="h", space="PSUM")
        nc.tensor.matmul(ph2[:, :W], w2[:], h1[:, :W], start=True, stop=True)
        h2 = sb_pool.tile([128, MAXBLK * F], F32, tag="h2")
        nc.scalar.activation(out=h2[:, :W], in_=ph2[:, :W],
                             func=mybir.ActivationFunctionType.Relu,
                             bias=b2[:, 0:1], scale=1.0)

        # ---- transpose back to token-major
        tp_y = ps_pool.tile([128, MAXBLK * F], F32, tag="y", space="PSUM")
        for b in range(nb):
            nc.tensor.transpose(out=tp_y[:, b * F:(b + 1) * F],
                                in_=h2[:, b * F:(b + 1) * F], identity=ident)
        y4 = sb_pool.tile([128, MAXBLK * F], F32, tag="y4")
        nc.scalar.activation(out=y4[:, :W], in_=tp_y[:, :W],
                             func=mybir.ActivationFunctionType.Copy)

        # ---- LayerNorm per 128-token block over F
        for b in range(nb):
            yb = y4[:, b * F:(b + 1) * F]
            stats = ln_pool.tile([128, 6], F32, tag="stats")
            nc.vector.bn_stats(out=stats[:], in_=yb)
            mv = ln_pool.tile([128, 2], F32, tag="mv")
            nc.vector.bn_aggr(out=mv[:], in_=stats[:])
            nc.scalar.activation(out=mv[:, 1:2], in_=mv[:, 1:2],
                                 func=mybir.ActivationFunctionType.Sqrt,
                                 bias=eps_sb[:, 0:1], scale=1.0)
            nc.vector.reciprocal(out=mv[:, 1:2], in_=mv[:, 1:2])
            nc.vector.tensor_scalar(out=yb, in0=yb,
                                    scalar1=mv[:, 0:1], scalar2=mv[:, 1:2],
                                    op0=mybir.AluOpType.subtract,
                                    op1=mybir.AluOpType.mult)
            nc.vector.tensor_mul(yb, yb, gam_bc[:])
            nc.vector.tensor_add(yb, yb, bet_bc[:])

        nc.sync.dma_start(
            out=o_r[:, sb * MAXBLK:sb * MAXBLK + nb, :],
            in_=y4[:, :W].rearrange("p (b f) -> p b f", f=F))


def build_nc(cfg=None):
    g = _geom(cfg or CFG)
    NC, E, K, F = g["NC"], g["E"], g["K"], g["F"]

    nc = bacc.Bacc("TRN2", target_bir_lowering=False, debug=False,
                   enable_asserts=False, num_devices=NC)

    # --- external I/O (per core)
    eidx = nc.dram_tensor("eidx", [128, g["E_BLK"] * K], I32,
                          kind="ExternalInput").ap()
    nidx = nc.dram_tensor("nidx", [128, g["N_BLK"] * K], I32,
                          kind="ExternalInput").ap()
    xe = nc.dram_tensor("xe", [g["E_PAD"], F], F32, kind="ExternalInput").ap()
    xn = nc.dram_tensor("xn", [g["N_PAD"], F], F32, kind="ExternalInput").ap()
    wts = {}
    for nm, shp in [("we1a", [F, F]), ("we1b", [F, F]), ("be1", [F]),
                    ("we2", [F, F]), ("be2", [F]),
                    ("wn1a", [F, F]), ("wn1b", [F, F]), ("bn1", [F]),
                    ("wn2", [F, F]), ("bn2", [F]),
                    ("gam", [F]), ("bet", [F])]:
        wts[nm] = nc.dram_tensor(nm, shp, F32, kind="ExternalInput").ap()
    oute = nc.dram_tensor("oute", [g["E_PAD"], F], F32,
                          kind="ExternalOutput").ap()
    outn = nc.dram_tensor("outn", [g["N_PAD"], F], F32,
                          kind="ExternalOutput").ap()

    with tile.TileContext(nc) as tc:
        with (
            tc.tile_pool(name="const", bufs=1) as const_pool,
            tc.tile_pool(name="sb", bufs=2) as sb_pool,
            tc.tile_pool(name="psum", bufs=2, space="PSUM") as ps_pool,
            tc.tile_pool(name="ln", bufs=4) as ln_pool,
            tc.tile_pool(name="dram", bufs=1, space="DRAM") as dram_pool,
        ):
            pools = dict(sb=sb_pool, psum=ps_pool, ln=ln_pool)

            # --- one-time constants
            ident = const_pool.tile([128, 128], F32)
            make_identity(nc, ident[:])
            eps_sb = const_pool.tile([128, 1], F32)
            nc.vector.memset(eps_sb[:], 1e-5)

            def load_w(name):
                t = const_pool.tile([F, F], F32, name=f"sb_{name}")
                nc.sync.dma_start(out=t[:], in_=wts[name][:])
                return t

            def load_b(name):
                t = const_pool.tile([128, 1], F32, name=f"sb_{name}")
                nc.sync.dma_start(out=t[:], in_=wts[name][:, None])
                return t

            def load_bc(name):
                t = const_pool.tile([128, F], F32, name=f"sb_{name}")
                src = bass.AP(tensor=wts[name].tensor, offset=0,
                              ap=[[0, 128], [1, F]])
                nc.gpsimd.dma_start(out=t[:], in_=src)
                return t

            we1a, we1b, we2 = load_w("we1a"), load_w("we1b"), load_w("we2")
            wn1a, wn1b, wn2 = load_w("wn1a"), load_w("wn1b"), load_w("wn2")
            be1, be2 = load_b("be1"), load_b("be2")
            bn1, bn2 = load_b("bn1"), load_b("bn2")
            gam_bc, bet_bc = load_bc("gam"), load_bc("bet")

            # --- resident index tiles (host pre-shuffled to partition-major)
            eidx_sb = const_pool.tile([128, g["E_BLK"] * K], I32)
            nc.sync.dma_start(out=eidx_sb[:], in_=eidx[:])
            nidx_sb = const_pool.tile([128, g["N_BLK"] * K], I32)
            nc.sync.dma_start(out=nidx_sb[:], in_=nidx[:])

            # --- internal DRAM
            exe_loc = dram_pool.tile([g["E_SH"], F], F32)
            tbl_e = dram_pool.tile([E, F], F32, addr_space="Shared")
            eloc = dram_pool.tile([g["E_PAD"], F], F32)
            tbl_new = dram_pool.tile([E, F], F32, addr_space="Shared")

            # --- assemble the full edge-feat table on device (input halo)
            nc.sync.dma_start(out=exe_loc[:], in_=xe[:g["E_SH"], :])
            nc.gpsimd.collective_compute(
                "AllGather", mybir.AluOpType.bypass,
                replica_groups=[list(range(NC))],
                ins=[exe_loc[:].opt()],
                outs=[tbl_e[:].opt()],
            )

            # --- edge path
            _emit_path(nc, tc, pools, tbl=tbl_e[:], idx_sb=eidx_sb[:],
                       x_dram=xe, out_dram=eloc[:],
                       w1a=we1a, w1b=we1b, b1=be1, w2=we2, b2=be2,
                       gam_bc=gam_bc, bet_bc=bet_bc, eps_sb=eps_sb,
                       ident=ident, n_blocks=g["E_BLK"], g=g)

            # copy local shard to external output (overlaps with node path)
            nc.sync.dma_start(out=oute[:g["E_SH"], :],
                              in_=eloc[:g["E_SH"], :])

            # --- all-gather the new edge features across the 8 cores
            nc.gpsimd.collective_compute(
                "AllGather", mybir.AluOpType.bypass,
                replica_groups=[list(range(NC))],
                ins=[eloc[:g["E_SH"], :].opt()],
                outs=[tbl_new[:].opt()],
            )

            # --- node path (gathers from the all-gathered table)
            _emit_path(nc, tc, pools, tbl=tbl_new[:], idx_sb=nidx_sb[:],
                       x_dram=xn, out_dram=outn,
                       w1a=wn1a, w1b=wn1b, b1=bn1, w2=wn2, b2=bn2,
                       gam_bc=gam_bc, bet_bc=bet_bc, eps_sb=eps_sb,
                       ident=ident, n_blocks=g["N_BLK"], g=g)

    nc.finalize()
    return nc, g


_BUILT = None


def _shuffle_idx(idx, g):
    """[rows, K] int32 -> [128, nblk*K] partition-major for the gather AP."""
    nblk = idx.shape[0] // 128
    return (idx.reshape(nblk, 128, g["K"]).transpose(1, 0, 2)
            .reshape(128, nblk * g["K"]).copy())


def _pad_rows(a, n):
    if a.shape[0] == n:
        return np.ascontiguousarray(a)
    out = np.zeros((n,) + a.shape[1:], a.dtype)
    out[:a.shape[0]] = a
    return out


def _make_in_maps(inputs, g):
    K, F = g["K"], g["F"]
    f32 = lambda x: np.ascontiguousarray(np.asarray(x), dtype=np.float32)
    i32 = lambda x: np.ascontiguousarray(np.asarray(x), dtype=np.int32)

    e_nb = i32(inputs["edge_neighbors"])
    n_nb = i32(inputs["node_neighbors"])
    ef = f32(inputs["edge_feats"])
    nf = f32(inputs["node_feats"])

    w_e1 = f32(inputs["W_edge_agg"])
    w_n1 = f32(inputs["W_node_agg"])
    shared = dict(
        we1a=np.ascontiguousarray(w_e1[:F]),
        we1b=np.ascontiguousarray(w_e1[F:] / K),
        be1=f32(inputs["b_edge_agg"]),
        we2=f32(inputs["W_edge_com"]), be2=f32(inputs["b_edge_com"]),
        wn1a=np.ascontiguousarray(w_n1[:F]),
        wn1b=np.ascontiguousarray(w_n1[F:] / K),
        bn1=f32(inputs["b_node_agg"]),
        wn2=f32(inputs["W_node_com"]), bn2=f32(inputs["b_node_com"]),
        gam=f32(inputs["ln_gamma"]), bet=f32(inputs["ln_beta"]),
    )

    in_maps = []
    for c in range(g["NC"]):
        esl = slice(c * g["E_SH"], (c + 1) * g["E_SH"])
        nsl = slice(c * g["N_SH"], (c + 1) * g["N_SH"])
        m = dict(shared)
        m["eidx"] = _shuffle_idx(_pad_rows(e_nb[esl], g["E_PAD"]), g)
        m["nidx"] = _shuffle_idx(_pad_rows(n_nb[nsl], g["N_PAD"]), g)
        m["xe"] = _pad_rows(ef[esl], g["E_PAD"])
        m["xn"] = _pad_rows(nf[nsl], g["N_PAD"])
        in_maps.append(m)
    return in_maps


def _run(inputs, cfg=None, trace=False):
    global _BUILT
    cfg = cfg or CFG
    if _BUILT is None or _BUILT[2] != cfg:
        nc, g = build_nc(cfg)
        _BUILT = (nc, g, dict(cfg))
    nc, g, _ = _BUILT
    NC = g["NC"]

    in_maps = _make_in_maps(inputs, g)
    try:
        res = run_bass_kernel_spmd(nc, in_maps, list(range(NC)), trace=trace)
    except Exception:
        # one retry for transient device/tunnel flakes
        res = run_bass_kernel_spmd(nc, in_maps, list(range(NC)), trace=trace)

    new_edge = np.concatenate(
        [res.results[c]["oute"][:g["E_SH"]] for c in range(NC)], axis=0)
    new_node = np.concatenate(
        [res.results[c]["outn"][:g["N_SH"]] for c in range(NC)], axis=0)
    return (new_node, new_edge), res


def kernel(**inputs):
    (new_node, new_edge), _ = _run(inputs)
    return new_node, new_edge


# revision 9
# speedup vs baseline: 2.1944x; 1.1876x over previous
"""CAGNN layer (edge+node message passing) on 8 trn2 NeuronCores.

Sharding: edges and nodes row-sharded across the 8 cores. The full edge-feat
gather table is assembled on-device by an AllGather of the per-core shards;
new edge features are exchanged with a second AllGather (the halo exchange)
before the node-path gather.

Per-core dataflow (both paths share the same structure):
  indirect-DMA gather of 16 neighbor rows per token (16 x 64KB per 128-token
  block) -> DVE tree-sum over the 16 neighbors
  -> PE transpose to feature-major
  -> matmul [self|neigh-sum] @ W  (neigh half of W pre-scaled by 1/16 on host)
  -> ReLU (+bias) on ACT, second matmul, ReLU
  -> PE transpose back to token-major
  -> LayerNorm (bn_stats/bn_aggr on DVE)
  -> DMA out
"""

import numpy as np

import concourse.bacc as bacc
import concourse.bass as bass
import concourse.tile as tile
from concourse import mybir
from concourse.bass_utils import run_bass_kernel_spmd
from concourse.masks import make_identity

F32 = mybir.dt.float32
I32 = mybir.dt.int32

# Problem geometry (full size; overridable for small-scale testing).
CFG = dict(
    NC=8,          # cores
    E=100000,      # edges
    N=50000,       # nodes
    K=16,          # neighbors
    F=128,         # feature/hidden dim (all of Fn, Fe, H)
)

MAXBLK = 4  # 128-token blocks per superblock (PSUM free-dim limit: 512 f32)


def _geom(cfg):
    NC, E, N, K, F = (cfg[k] for k in ("NC", "E", "N", "K", "F"))
    e_sh = E // NC
    n_sh = N // NC
    e_blk = -(-e_sh // 128)          # 128-token blocks per core, edge path
    n_blk = -(-n_sh // 128)
    return dict(NC=NC, E=E, N=N, K=K, F=F,
                E_SH=e_sh, N_SH=n_sh, E_BLK=e_blk, N_BLK=n_blk,
                E_PAD=e_blk * 128, N_PAD=n_blk * 128)


def _emit_path(nc, tc, pools, *, tbl, idx_sb, x_dram, out_dram,
               w1a, w1b, b1, w2, b2, gam_bc, bet_bc, eps_sb, ident,
               n_blocks, g):
    """Emit one message-passing path (edge or node) over n_blocks blocks of
    128 tokens, grouped into superblocks of up to MAXBLK blocks.

    tbl:    DRAM gather table [T, F]
    idx_sb: resident SBUF index tile [128, n_blocks*K] int32 (partition-major)
    x_dram: self features [n_blocks*128, F] DRAM
    out_dram: output [n_blocks*128, F] DRAM
    """
    K, F = g["K"], g["F"]
    sb_pool, ps_pool, ln_pool = pools["sb"], pools["psum"], pools["ln"]

    x_r = x_dram.rearrange("(j p) f -> p j f", p=128)
    o_r = out_dram.rearrange("(j p) f -> p j f", p=128)
    idx3 = idx_sb.rearrange("p (j k) -> p j k", k=K)

    n_sb = -(-n_blocks // MAXBLK)
    for sb in range(n_sb):
        nb = min(MAXBLK, n_blocks - sb * MAXBLK)
        W = nb * F  # free width this superblock

        # ---- neighbor gather + tree-sum -> s4 [128, W] (sum of K rows)
        s4 = sb_pool.tile([128, MAXBLK * F], F32, tag="s4")
        for b in range(nb):
            j = sb * MAXBLK + b
            gt = sb_pool.tile([128, K * F], F32, tag="gather", bufs=10)
            for k in range(K):
                nc.gpsimd.indirect_dma_start(
                    out=gt[:, k * F:(k + 1) * F],
                    out_offset=None,
                    in_=tbl[:],
                    in_offset=bass.IndirectOffsetOnAxis(
                        ap=idx3[:, j, k:k + 1], axis=0),
                )
            w = K * F // 2
            while w >= F:
                lo = gt[:, :w]
                hi = gt[:, w:2 * w]
                if w == F:
                    nc.vector.tensor_add(s4[:, b * F:(b + 1) * F], lo, hi)
                else:
                    nc.vector.tensor_add(lo, lo, hi)
                w //= 2

        # ---- self features
        x4 = sb_pool.tile([128, MAXBLK, F], F32, tag="x4")
        nc.sync.dma_start(out=x4[:, :nb, :],
                          in_=x_r[:, sb * MAXBLK:sb * MAXBLK + nb, :])

        # ---- transpose self + neigh-sum to feature-major
        tp_x = ps_pool.tile([128, MAXBLK * F], F32, tag="tp", space="PSUM")
        for b in range(nb):
            nc.tensor.transpose(out=tp_x[:, b * F:(b + 1) * F],
                                in_=x4[:, b, :], identity=ident)
        xt = sb_pool.tile([128, MAXBLK * F], F32, tag="xt")
        nc.scalar.activation(out=xt[:, :W], in_=tp_x[:, :W],
                             func=mybir.ActivationFunctionType.Copy)

        tp_s = ps_pool.tile([128, MAXBLK * F], F32, tag="tp", space="PSUM")
        for b in range(nb):
            nc.tensor.transpose(out=tp_s[:, b * F:(b + 1) * F],
                                in_=s4[:, b * F:(b + 1) * F], identity=ident)
        st = sb_pool.tile([128, MAXBLK * F], F32, tag="st")
        nc.scalar.activation(out=st[:, :W], in_=tp_s[:, :W],
                             func=mybir.ActivationFunctionType.Copy)

        # ---- h1 = relu(W1a.T @ xt + W1b.T @ st + b1)   [H, W]
        ph1 = ps_pool.tile([128, MAXBLK * F], F32, tag="h", space="PSUM")
        nc.tensor.matmul(ph1[:, :W], w1a[:], xt[:, :W], start=True, stop=False)
        nc.tensor.matmul(ph1[:, :W], w1b[:], st[:, :W], start=False, stop=True)
        h1 = sb_pool.tile([128, MAXBLK * F], F32, tag="h1")
        nc.scalar.activation(out=h1[:, :W], in_=ph1[:, :W],
                             func=mybir.ActivationFunctionType.Relu,
                             bias=b1[:, 0:1], scale=1.0)

        # ---- h2 = relu(W2.T @ h1 + b2)   [H, W]
        ph2 = ps_pool.tile([128, MAXBLK * F], F32, tag="h", space="PSUM")
        nc.tensor.matmul(ph2[:, :W], w2[:], h1[:, :W], start=True, stop=True)
        h2 = sb_pool.tile([128, MAXBLK * F], F32, tag="h2")
        nc.scalar.activation(out=h2[:, :W], in_=ph2[:, :W],
                             func=mybir.ActivationFunctionType.Relu,
                             bias=b2[:, 0:1], scale=1.0)

        # ---- transpose back to token-major
        tp_y = ps_pool.tile([128, MAXBLK * F], F32, tag="y", space="PSUM")
        for b in range(nb):
            nc.tensor.transpose(out=tp_y[:, b * F:(b + 1) * F],
                                in_=h2[:, b * F:(b + 1) * F], identity=ident)
        y4 = sb_pool.tile([128, MAXBLK * F], F32, tag="y4")
        nc.scalar.activation(out=y4[:, :W], in_=tp_y[:, :W],
                             func=mybir.ActivationFunctionType.Copy)

        # ---- LayerNorm per 128-token block over F
        for b in range(nb):
            yb = y4[:, b * F:(b + 1) * F]
            stats = ln_pool.tile([128, 6], F32, tag="stats")
            nc.vector.bn_stats(out=stats[:], in_=yb)
            mv = ln_pool.tile([128, 2], F32, tag="mv")
            nc.vector.bn_aggr(out=mv[:], in_=stats[:])
            nc.scalar.activation(out=mv[:, 1:2], in_=mv[:, 1:2],
                                 func=mybir.ActivationFunctionType.Sqrt,
                                 bias=eps_sb[:, 0:1], scale=1.0)
            nc.vector.reciprocal(out=mv[:, 1:2], in_=mv[:, 1:2])
            nc.vector.tensor_scalar(out=yb, in0=yb,
                                    scalar1=mv[:, 0:1], scalar2=mv[:, 1:2],
                                    op0=mybir.AluOpType.subtract,
                                    op1=mybir.AluOpType.mult)
            nc.vector.tensor_mul(yb, yb, gam_bc[:])
            nc.vector.tensor_add(yb, yb, bet_bc[:])

        nc.sync.dma_start(
            out=o_r[:, sb * MAXBLK:sb * MAXBLK + nb, :],
            in_=y4[:, :W].rearrange("p (b f) -> p b f", f=F))


def build_nc(cfg=None):
    g = _geom(cfg or CFG)
    NC, E, K, F = g["NC"], g["E"], g["K"], g["F"]

    nc = bacc.Bacc("TRN2", target_bir_lowering=False, debug=False,
                   enable_asserts=False, num_devices=NC)

    # --- external I/O (per core)
    eidx = nc.dram_tensor("eidx", [128, g["E_BLK"] * K], I32,
                          kind="ExternalInput").ap()
    nidx = nc.dram_tensor("nidx", [128, g["N_BLK"] * K], I32,
                          kind="ExternalInput").ap()
    xe = nc.dram_tensor("xe", [g["E_PAD"], F], F32, kind="ExternalInput").ap()
    xn = nc.dram_tensor("xn", [g["N_PAD"], F], F32, kind="ExternalInput").ap()
    wts = {}
    for nm, shp in [("we1a", [F, F]), ("we1b", [F, F]), ("be1", [F]),
                    ("we2", [F, F]), ("be2", [F]),
                    ("wn1a", [F, F]), ("wn1b", [F, F]), ("bn1", [F]),
                    ("wn2", [F, F]), ("bn2", [F]),
                    ("gam", [F]), ("bet", [F])]:
        wts[nm] = nc.dram_tensor(nm, shp, F32, kind="ExternalInput").ap()
    oute = nc.dram_tensor("oute", [g["E_PAD"], F], F32,
                          kind="ExternalOutput").ap()
    outn = nc.dram_tensor("outn", [g["N_PAD"], F], F32,
                          kind="ExternalOutput").ap()

    with tile.TileContext(nc) as tc:
        with (
            tc.tile_pool(name="const", bufs=1) as const_pool,
            tc.tile_pool(name="sb", bufs=2) as sb_pool,
            tc.tile_pool(name="psum", bufs=2, space="PSUM") as ps_pool,
            tc.tile_pool(name="ln", bufs=4) as ln_pool,
            tc.tile_pool(name="dram", bufs=1, space="DRAM") as dram_pool,
        ):
            pools = dict(sb=sb_pool, psum=ps_pool, ln=ln_pool)

            # --- one-time constants
            ident = const_pool.tile([128, 128], F32)
            make_identity(nc, ident[:])
            eps_sb = const_pool.tile([128, 1], F32)
            nc.vector.memset(eps_sb[:], 1e-5)

            def load_w(name):
                t = const_pool.tile([F, F], F32, name=f"sb_{name}")
                nc.sync.dma_start(out=t[:], in_=wts[name][:])
                return t

            def load_b(name):
                t = const_pool.tile([128, 1], F32, name=f"sb_{name}")
                nc.sync.dma_start(out=t[:], in_=wts[name][:, None])
                return t

            def load_bc(name):
                t = const_pool.tile([128, F], F32, name=f"sb_{name}")
                src = bass.AP(tensor=wts[name].tensor, offset=0,
                              ap=[[0, 128], [1, F]])
                nc.gpsimd.dma_start(out=t[:], in_=src)
                return t

            we1a, we1b, we2 = load_w("we1a"), load_w("we1b"), load_w("we2")
            wn1a, wn1b, wn2 = load_w("wn1a"), load_w("wn1b"), load_w("wn2")
            be1, be2 = load_b("be1"), load_b("be2")
            bn1, bn2 = load_b("bn1"), load_b("bn2")
            gam_bc, bet_bc = load_bc("gam"), load_bc("bet")

            # --- resident index tiles (host pre-shuffled to partition-major)
            eidx_sb = const_pool.tile([128, g["E_BLK"] * K], I32)
            nc.sync.dma_start(out=eidx_sb[:], in_=eidx[:])
            nidx_sb = const_pool.tile([128, g["N_BLK"] * K], I32)
            nc.sync.dma_start(out=nidx_sb[:], in_=nidx[:])

            # --- internal DRAM
            exe_loc = dram_pool.tile([g["E_SH"], F], F32)
            tbl_e = dram_pool.tile([E, F], F32, addr_space="Shared")
            eloc = dram_pool.tile([g["E_PAD"], F], F32)
            tbl_new = dram_pool.tile([E, F], F32, addr_space="Shared")

            # --- assemble the full edge-feat table on device (input halo)
            nc.sync.dma_start(out=exe_loc[:], in_=xe[:g["E_SH"], :])
            nc.gpsimd.collective_compute(
                "AllGather", mybir.AluOpType.bypass,
                replica_groups=[list(range(NC))],
                ins=[exe_loc[:].opt()],
                outs=[tbl_e[:].opt()],
            )

            # --- edge path
            _emit_path(nc, tc, pools, tbl=tbl_e[:], idx_sb=eidx_sb[:],
                       x_dram=xe, out_dram=eloc[:],
                       w1a=we1a, w1b=we1b, b1=be1, w2=we2, b2=be2,
                       gam_bc=gam_bc, bet_bc=bet_bc, eps_sb=eps_sb,
                       ident=ident, n_blocks=g["E_BLK"], g=g)

            # copy local shard to external output (overlaps with node path)
            nc.sync.dma_start(out=oute[:g["E_SH"], :],
                              in_=eloc[:g["E_SH"], :])

            # --- all-gather the new edge features across the 8 cores
            nc.gpsimd.collective_compute(
                "AllGather", mybir.AluOpType.bypass,
                replica_groups=[list(range(NC))],
                ins=[eloc[:g["E_SH"], :].opt()],
                outs=[tbl_new[:].opt()],
            )

            # --- node path (gathers from the all-gathered table)
            _emit_path(nc, tc, pools, tbl=tbl_new[:], idx_sb=nidx_sb[:],
                       x_dram=xn, out_dram=outn,
                       w1a=wn1a, w1b=wn1b, b1=bn1, w2=wn2, b2=bn2,
                       gam_bc=gam_bc, bet_bc=bet_bc, eps_sb=eps_sb,
                       ident=ident, n_blocks=g["N_BLK"], g=g)

    nc.finalize()
    return nc, g


_BUILT = None


def _shuffle_idx(idx, g):
    """[rows, K] int32 -> [128, nblk*K] partition-major for the gather AP."""
    nblk = idx.shape[0] // 128
    return (idx.reshape(nblk, 128, g["K"]).transpose(1, 0, 2)
            .reshape(128, nblk * g["K"]).copy())


def _pad_rows(a, n):
    if a.shape[0] == n:
        return np.ascontiguousarray(a)
    out = np.zeros((n,) + a.shape[1:], a.dtype)
    out[:a.shape[0]] = a
    return out


def _make_in_maps(inputs, g):
    K, F = g["K"], g["F"]
    f32 = lambda x: np.ascontiguousarray(np.asarray(x), dtype=np.float32)
    i32 = lambda x: np.ascontiguousarray(np.asarray(x), dtype=np.int32)

    e_nb = i32(inputs["edge_neighbors"])
    n_nb = i32(inputs["node_neighbors"])
    ef = f32(inputs["edge_feats"])
    nf = f32(inputs["node_feats"])

    w_e1 = f32(inputs["W_edge_agg"])
    w_n1 = f32(inputs["W_node_agg"])
    shared = dict(
        we1a=np.ascontiguousarray(w_e1[:F]),
        we1b=np.ascontiguousarray(w_e1[F:] / K),
        be1=f32(inputs["b_edge_agg"]),
        we2=f32(inputs["W_edge_com"]), be2=f32(inputs["b_edge_com"]),
        wn1a=np.ascontiguousarray(w_n1[:F]),
        wn1b=np.ascontiguousarray(w_n1[F:] / K),
        bn1=f32(inputs["b_node_agg"]),
        wn2=f32(inputs["W_node_com"]), bn2=f32(inputs["b_node_com"]),
        gam=f32(inputs["ln_gamma"]), bet=f32(inputs["ln_beta"]),
    )

    in_maps = []
    for c in range(g["NC"]):
        esl = slice(c * g["E_SH"], (c + 1) * g["E_SH"])
        nsl = slice(c * g["N_SH"], (c + 1) * g["N_SH"])
        m = dict(shared)
        m["eidx"] = _shuffle_idx(_pad_rows(e_nb[esl], g["E_PAD"]), g)
        m["nidx"] = _shuffle_idx(_pad_rows(n_nb[nsl], g["N_PAD"]), g)
        m["xe"] = _pad_rows(ef[esl], g["E_PAD"])
        m["xn"] = _pad_rows(nf[nsl], g["N_PAD"])
        in_maps.append(m)
    return in_maps


def _run(inputs, cfg=None, trace=False):
    global _BUILT
    cfg = cfg or CFG
    if _BUILT is None or _BUILT[2] != cfg:
        nc, g = build_nc(cfg)
        _BUILT = (nc, g, dict(cfg))
    nc, g, _ = _BUILT
    NC = g["NC"]

    in_maps = _make_in_maps(inputs, g)
    try:
        res = run_bass_kernel_spmd(nc, in_maps, list(range(NC)), trace=trace)
    except Exception:
        # one retry for transient device/tunnel flakes
        res = run_bass_kernel_spmd(nc, in_maps, list(range(NC)), trace=trace)

    new_edge = np.concatenate(
        [res.results[c]["oute"][:g["E_SH"]] for c in range(NC)], axis=0)
    new_node = np.concatenate(
        [res.results[c]["outn"][:g["N_SH"]] for c in range(NC)], axis=0)
    return (new_node, new_edge), res


def kernel(**inputs):
    (new_node, new_edge), _ = _run(inputs)
    return new_node, new_edge
